# revision 1
# baseline (speedup 1.0000x reference)
"""Distributed Trainium2 kernel: LayerNorm -> QKV -> causal MHA -> out-proj.

Sharding (8 cores):
  - LayerNorm + final projection: token-parallel (4096 tokens -> 512/core).
  - Attention + QKV: head-parallel (16 heads -> 2/core).
  - Comms: AllGather of xn^T (bf16) after LN; AllToAll of per-head attention
    output before the final projection (re-shards the inner-dim contraction
    to token-parallel; no AllReduce needed).

Layout notes:
  - All activations are kept TRANSPOSED ([feature, token]) so every matmul
    contraction runs over the partition axis.  S is computed transposed
    (S^T[j,i] = k_j . q_i); softmax sums come from an appended ones-column
    on V (m=65 matmul); causal masking is a multiplicative 0/1 bf16 mask on
    exp(S^T) tiles; the dh^-0.5 scale rides the exp activation's free affine.
  - Matmul inputs are bf16 (4x the fp32 TensorE rate); accumulation fp32.
    Weights arrive pre-cast to bf16 (host-side layout prep).
  - Attention groups are interleaved into the QKV token-chunk loop (group
    (b, ic) right after its last prerequisite chunk) so ScalarE exp work
    overlaps TensorE QKV matmuls and the PE stream stays dense (HAM-warm).
"""

import numpy as np
import ml_dtypes

import concourse.bass as bass
import concourse.tile as tile
from concourse import bacc, mybir
from concourse.bass import ds, ts
from concourse.bass_utils import run_bass_kernel_spmd
from concourse.masks import make_identity

B, N, D = 2, 2048, 1024
HEADS, DH = 16, 64
INNER = HEADS * DH          # 1024
NCORES = 8
T = B * N                   # 4096 tokens
TS = T // NCORES            # 512 tokens per core
HPC = HEADS // NCORES       # 2 heads per core
SCALE = float(DH) ** -0.5   # 0.125
EPS = 1e-5

FP = mybir.dt.float32
BF = mybir.dt.bfloat16

KT = D // 128               # 8 contraction tiles of 128 over dim
TCH = T // 512              # 8 token chunks of 512 (== ranks)
ICB = N // 512              # 4 i-chunks of 512 per batch
JTB = N // 128              # 16 j-tiles of 128 per batch


def build():
    nc = bacc.Bacc("TRN2", target_bir_lowering=False, debug=False,
                   num_devices=NCORES)

    x_sh = nc.dram_tensor("x_shard", [TS, D], FP, kind="ExternalInput")
    gamma_t = nc.dram_tensor("gamma", [D], FP, kind="ExternalInput")
    beta_t = nc.dram_tensor("beta", [D], FP, kind="ExternalInput")
    wq_t = nc.dram_tensor("wq", [D, HPC * DH], BF, kind="ExternalInput")
    wk_t = nc.dram_tensor("wk", [D, HPC * DH], BF, kind="ExternalInput")
    wv_t = nc.dram_tensor("wv", [D, HPC * DH], BF, kind="ExternalInput")
    wo_t = nc.dram_tensor("w_out", [INNER, D], BF, kind="ExternalInput")
    out_sh = nc.dram_tensor("out_shard", [TS, D], FP, kind="ExternalOutput")

    with tile.TileContext(nc) as tc:
        _body(nc, tc, x_sh, gamma_t, beta_t, wq_t, wk_t, wv_t, wo_t, out_sh)

    nc.compile()
    return nc


def _att_thunks(nc, b, ic, kTt, qT, vhat, masks, outT,
                s_ps, av_ps, espool, smallp):
    """Attention for query chunk (b, ic) as a list of PE-ordered thunks.

    Pipelined: S-matmuls for step jp are emitted before the AV-matmuls of
    step jp-1, so the PE never sits directly behind the ACT exp latency.
    Each thunk emits its own ACT/DVE companions (exp, mask, normalize).
    """
    q_idx = b * ICB + ic
    njt = 4 * (ic + 1)
    av = [av_ps.tile([128, 512], FP, tag=f"av{h}", name=f"av{h}_{q_idx}")
          for h in range(HPC)]
    es = {}

    def spair(h, jp):
        def run():
            sx = s_ps.tile([128, 1024], FP, tag="sx",
                           name=f"sx{h}_{q_idx}_{jp}")
            e = espool.tile([128, 1024], BF, tag="es",
                            name=f"es{h}_{q_idx}_{jp}")
            es[(h, jp)] = e
            for u in range(2):
                jt = 2 * jp + u
                tq = b * ICB + jt // 4
                jo = 128 * (jt % 4)
                nc.tensor.matmul(
                    sx[:, ds(512 * u, 512)],
                    kTt[ds(64 * h, 64), tq, ds(jo, 128)],
                    qT[ds(64 * h, 64), q_idx, :],
                    start=True, stop=True,
                    tile_position=(64 * h, 0))
            nc.scalar.activation(
                out=e, in_=sx,
                func=mybir.ActivationFunctionType.Exp, scale=SCALE)
        return run

    def avpair(h, jp):
        def run():
            e = es.pop((h, jp))
            for u in range(2):
                jt = 2 * jp + u
                m = jt - 4 * ic
                if 0 <= m < 4:
                    nc.vector.tensor_tensor(
                        out=e[:, ds(512 * u, 512)],
                        in0=e[:, ds(512 * u, 512)],
                        in1=masks[:, m, :],
                        op=mybir.AluOpType.mult)
                nc.tensor.matmul(
                    av[h][0:65, :],
                    vhat[:, b * JTB + jt, ds(65 * h, 65)],
                    e[:, ds(512 * u, 512)],
                    start=(jt == 0), stop=(jt == njt - 1))
        return run

    def norm(h):
        def run():
            rsum = smallp.tile([1, 512], FP, tag="rsum", name=f"rs{h}_{q_idx}")
            nc.vector.tensor_copy(out=rsum, in_=av[h][64:65, :])
            rec = smallp.tile([1, 512], FP, tag="rec", name=f"rc{h}_{q_idx}")
            nc.vector.reciprocal_approx_fast(out=rec, in_=rsum)
            bc = smallp.tile([64, 512], FP, tag="bc", name=f"bc{h}_{q_idx}")
            nc.gpsimd.partition_broadcast(bc, rec)
            nc.vector.tensor_tensor(
                out=outT[h][:, ds(512 * q_idx, 512)],
                in0=av[h][0:64, :], in1=bc,
                op=mybir.AluOpType.mult)
        return run

    thunks = []
    nps = njt // 2
    thunks.append(spair(0, 0))
    thunks.append(spair(1, 0))
    for jp in range(1, nps):
        thunks.append(spair(0, jp))
        thunks.append(avpair(0, jp - 1))
        thunks.append(spair(1, jp))
        thunks.append(avpair(1, jp - 1))
    thunks.append(avpair(0, nps - 1))
    thunks.append(avpair(1, nps - 1))
    thunks.append(norm(0))
    thunks.append(norm(1))
    return thunks


def _qkv_thunks(nc, tci, xt_full, wq_sb, wk_sb, wv_sb, qT, kTt, vhat,
                identity, qkv_ps, s_ps, vst):
    """QKV projection for token chunk tci as a list of PE-ordered thunks."""
    thunks = []

    def mk_group(w_sb, nm):
        acc = qkv_ps.tile([128, 512], FP, tag="acc", name=f"acc{nm}_{tci}")

        def mm(k):
            def run():
                nc.tensor.matmul(acc, w_sb[:, k, :], xt_full[:, k, :],
                                 start=(k == 0), stop=(k == KT - 1))
            return run
        return acc, mm

    for w_sb, dst, nm in ((wq_sb, qT, "q"), (wk_sb, kTt, "k")):
        acc, mm = mk_group(w_sb, nm)
        for k in range(KT):
            thunks.append(mm(k))

        def copy(acc=acc, dst=dst, tci=tci):
            nc.vector.tensor_copy(out=dst[:, tci, :], in_=acc)
        thunks.append(copy)

    accv, mmv = mk_group(wv_sb, "v")
    for k in range(KT):
        thunks.append(mmv(k))
    vs = vst.tile([128, 512], BF, tag="vs", name=f"vs_{tci}")

    def vcopy():
        nc.vector.tensor_copy(out=vs, in_=accv)
    thunks.append(vcopy)

    def vtrans():
        def run():
            vstg = vst.tile([128, 4, 128], BF, tag="vstg", name=f"vstg_{tci}")
            nc.sync.dma_start_transpose(out=vstg, in_=vs)
            nc.vector.tensor_copy(out=vhat[:, ds(tci * 4, 4), 0:64],
                                  in_=vstg[:, :, 0:64])
            nc.vector.tensor_copy(out=vhat[:, ds(tci * 4, 4), 65:129],
                                  in_=vstg[:, :, 64:128])
        return run
    thunks.append(vtrans())
    return thunks


def _merge(primary, filler):
    """Interleave filler thunks evenly between primary thunks."""
    out = []
    np_, nf = len(primary), len(filler)
    fi = 0
    for i, p in enumerate(primary):
        out.append(p)
        want = (i + 1) * nf // np_
        while fi < want:
            out.append(filler[fi])
            fi += 1
    out.extend(filler[fi:])
    return out


def _body(nc, tc, x_sh, gamma_t, beta_t, wq_t, wk_t, wv_t, wo_t, out_sh):
    from contextlib import ExitStack
    ctx = ExitStack()
    with ctx:
        const = ctx.enter_context(tc.tile_pool(name="const", bufs=1))
        wpool = ctx.enter_context(tc.tile_pool(name="wpool", bufs=1))
        big = ctx.enter_context(tc.tile_pool(name="big", bufs=1))
        dram = ctx.enter_context(tc.tile_pool(name="dram", bufs=1, space="DRAM"))

        # ---------- constants ----------
        identity = const.tile([128, 128], BF)
        make_identity(nc, identity)

        # 0/1 causal masks for the 4 diagonal block offsets (j0-i0 = 128*m):
        # valid iff 128*m + jj - ii <= 0, i.e. ii - jj - 128*m >= 0.
        masks = const.tile([128, 4, 512], BF)
        for m in range(4):
            nc.gpsimd.memset(masks[:, m, :], 1.0)
            nc.gpsimd.affine_select(
                out=masks[:, m, :], in_=masks[:, m, :],
                compare_op=mybir.AluOpType.is_ge, fill=0.0,
                base=-128 * m, pattern=[[1, 512]], channel_multiplier=-1,
            )

        eps_t = const.tile([128, 1], FP)
        nc.vector.memset(eps_t, EPS)

        gamma_b = const.tile([128, D], FP)
        g_ap = gamma_t.ap()
        nc.scalar.dma_start(out=gamma_b, in_=bass.AP(
            tensor=g_ap.tensor, offset=g_ap.offset,
            ap=[[0, 128]] + list(g_ap.ap)))
        beta_b = const.tile([128, D], BF)
        beta_f = const.tile([128, D], FP)
        b_ap = beta_t.ap()
        nc.scalar.dma_start(out=beta_f, in_=bass.AP(
            tensor=b_ap.tensor, offset=b_ap.offset,
            ap=[[0, 128]] + list(b_ap.ap)))
        nc.vector.tensor_copy(out=beta_b, in_=beta_f)

        # ---------- comm bounce buffers ----------
        ag_in = dram.tile([KT, 128, TS], BF)
        ag_out = dram.tile([NCORES, KT, 128, TS], BF, addr_space="Shared")
        a2a_in = dram.tile([NCORES, 128, TS], BF)
        a2a_out = dram.tile([NCORES, 128, TS], BF)

        wq_sb = wpool.tile([128, KT, HPC * DH], BF)
        wk_sb = wpool.tile([128, KT, HPC * DH], BF)
        wv_sb = wpool.tile([128, KT, HPC * DH], BF)
        wo_sb = wpool.tile([128, KT, D], BF)

        # ---------- phase A: LayerNorm + transpose -> ag_in ----------
        with tc.tile_pool(name="lnp", bufs=2) as lnp, \
             tc.tile_pool(name="lns", bufs=2) as lns, \
             tc.tile_pool(name="tstage", bufs=4) as tstage:
            nc.scalar.dma_start(
                out=wq_sb,
                in_=wq_t.ap().rearrange("(k p) c -> p k c", p=128))
            nc.scalar.dma_start(
                out=wk_sb,
                in_=wk_t.ap().rearrange("(k p) c -> p k c", p=128))
            nc.scalar.dma_start(
                out=wv_sb,
                in_=wv_t.ap().rearrange("(k p) c -> p k c", p=128))
            x_tiles = []
            for tt in range(TS // 128):
                x_t = lnp.tile([128, D], FP, tag=f"x{tt}", name=f"x_{tt}",
                               bufs=1)
                nc.sync.dma_start(out=x_t, in_=x_sh.ap()[ts(tt, 128), :])
                x_tiles.append(x_t)
            for tt in range(TS // 128):
                x_t = x_tiles[tt]
                stats = lns.tile([128, 2, 6], FP, tag="stats")
                xg = x_t.rearrange("p (s f) -> p s f", f=512)
                for s in range(2):
                    nc.vector.bn_stats(out=stats[:, s, :], in_=xg[:, s, :])
                mv = lns.tile([128, 2], FP, tag="mv")
                nc.vector.bn_aggr(out=mv, in_=stats)
                rstd = lns.tile([128, 1], FP, tag="rstd")
                nc.scalar.activation(out=rstd, in_=mv[:, 1:2],
                                     func=mybir.ActivationFunctionType.Sqrt,
                                     bias=eps_t, scale=1.0)
                nc.vector.reciprocal(out=rstd, in_=rstd)
                xn_f = lnp.tile([128, D], FP, tag="xnf")
                nc.vector.tensor_scalar(
                    out=xn_f, in0=x_t, scalar1=mv[:, 0:1], scalar2=rstd,
                    op0=mybir.AluOpType.subtract, op1=mybir.AluOpType.mult)
                xn_g = lnp.tile([128, D], BF, tag="xng")
                nc.vector.tensor_tensor(out=xn_g, in0=xn_f, in1=gamma_b,
                                        op=mybir.AluOpType.mult)
                xn_bf = lnp.tile([128, D], BF, tag="xnbf")
                nc.vector.tensor_tensor(out=xn_bf, in0=xn_g, in1=beta_b,
                                        op=mybir.AluOpType.add)
                xnT_s = tstage.tile([128, KT, 128], BF, tag="xnT")
                nc.scalar.dma_start_transpose(out=xnT_s, in_=xn_bf)
                nc.sync.dma_start(
                    out=ag_in[:, :, ts(tt, 128)].rearrange("k p t -> p k t"),
                    in_=xnT_s)

        nc.gpsimd.collective_compute(
            "AllGather", mybir.AluOpType.bypass,
            replica_groups=[list(range(NCORES))],
            ins=[ag_in.opt()], outs=[ag_out.opt()])

        # ---------- phase B+C: QKV projection + interleaved attention ----
        qT = big.tile([128, TCH, 512], BF)   # rows: [h0 64 | h1 64]
        kTt = big.tile([128, TCH, 512], BF)
        vhat = big.tile([128, JTB * B, 130], BF)  # [j, jt, 65*h+c], col 64/129 = 1
        nc.gpsimd.memset(vhat[:, :, 64:65], 1.0)
        nc.gpsimd.memset(vhat[:, :, 129:130], 1.0)
        outT = [big.tile([64, T], BF, name=f"outT{h}") for h in range(HPC)]

        with tc.tile_pool(name="xstream", bufs=3) as xstream, \
             tc.tile_pool(name="qkv_ps", bufs=2, space="PSUM") as qkv_ps, \
             tc.tile_pool(name="s_ps", bufs=2, space="PSUM") as s_ps, \
             tc.tile_pool(name="av_ps", bufs=1, space="PSUM") as av_ps, \
             tc.tile_pool(name="espool", bufs=6) as espool, \
             tc.tile_pool(name="smallp", bufs=4) as smallp, \
             tc.tile_pool(name="vstage", bufs=2) as vst:
            pending_att = None
            for tci in range(TCH):
                xt_full = xstream.tile([128, KT, 512], BF, tag="xt",
                                       name=f"xt_{tci}")
                nc.sync.dma_start(
                    out=xt_full,
                    in_=ag_out[tci].rearrange("k p t -> p k t"))
                if tci == 1:
                    nc.sync.dma_start(
                        out=wo_sb,
                        in_=wo_t.ap().rearrange("(k p) e -> p k e", p=128))
                qkv = _qkv_thunks(nc, tci, xt_full, wq_sb, wk_sb, wv_sb,
                                  qT, kTt, vhat, identity, qkv_ps, s_ps, vst)
                if pending_att is None:
                    seq = qkv
                else:
                    seq = _merge(pending_att, qkv)
                for thunk in seq:
                    thunk()
                b, ic = tci // ICB, tci % ICB
                pending_att = _att_thunks(nc, b, ic, kTt, qT, vhat, masks,
                                          outT, s_ps, av_ps, espool, smallp)
            for thunk in pending_att:
                thunk()

        # ---------- phase D: AllToAll + output projection ----------
        for r in range(NCORES):
            nc.sync.dma_start(out=a2a_in[r, 0:64, :],
                              in_=outT[0][:, ds(512 * r, 512)])
            nc.sync.dma_start(out=a2a_in[r, 64:128, :],
                              in_=outT[1][:, ds(512 * r, 512)])
        nc.gpsimd.collective_compute(
            "AllToAll", mybir.AluOpType.bypass,
            replica_groups=[list(range(NCORES))],
            ins=[a2a_in.opt()], outs=[a2a_out.opt()])

        a2a_sb = big.tile([128, NCORES, 512], BF)
        for r in range(NCORES):
            nc.sync.dma_start(out=a2a_sb[:, r, :], in_=a2a_out[r])

        out_view = out_sh.ap().rearrange("(t p) e -> p t e", p=128)
        with tc.tile_pool(name="op_ps", bufs=4, space="PSUM") as op_ps, \
             tc.tile_pool(name="ost", bufs=3) as ostp:
            for tt in range(TS // 128):
                for ec in range(D // 512):
                    po = op_ps.tile([128, 512], FP, tag="po")
                    for ct in range(NCORES):
                        nc.tensor.matmul(
                            po, a2a_sb[:, ct, ds(128 * tt, 128)],
                            wo_sb[:, ct, ds(512 * ec, 512)],
                            start=(ct == 0), stop=(ct == NCORES - 1))
                    ost = ostp.tile([128, 512], FP, tag="ost")
                    nc.vector.tensor_copy(out=ost, in_=po)
                    nc.sync.dma_start(out=out_view[:, tt, ds(512 * ec, 512)],
                                      in_=ost)


_NC = None
LAST_EXEC_TIME_NS = None


def _get_nc():
    global _NC
    if _NC is None:
        _NC = build()
    return _NC


def make_in_maps(x, gamma, beta, w_qkv, w_out):
    bf = ml_dtypes.bfloat16
    x = np.ascontiguousarray(np.asarray(x, dtype=np.float32)).reshape(T, D)
    gamma = np.ascontiguousarray(np.asarray(gamma, dtype=np.float32))
    beta = np.ascontiguousarray(np.asarray(beta, dtype=np.float32))
    w_qkv = np.asarray(w_qkv, dtype=np.float32).astype(bf)
    w_out = np.ascontiguousarray(np.asarray(w_out, dtype=np.float32).astype(bf))
    in_maps = []
    for c in range(NCORES):
        cols = slice(128 * c, 128 * c + 128)
        in_maps.append({
            "x_shard": np.ascontiguousarray(x[TS * c: TS * (c + 1)]),
            "gamma": gamma,
            "beta": beta,
            "wq": np.ascontiguousarray(w_qkv[:, cols]),
            "wk": np.ascontiguousarray(w_qkv[:, INNER:][:, cols]),
            "wv": np.ascontiguousarray(w_qkv[:, 2 * INNER:][:, cols]),
            "w_out": w_out,
        })
    return in_maps


def kernel(x, mask, gamma, beta, w_qkv, w_out):
    global LAST_EXEC_TIME_NS
    nc = _get_nc()
    in_maps = make_in_maps(x, gamma, beta, w_qkv, w_out)
    res = run_bass_kernel_spmd(nc, in_maps, core_ids=list(range(NCORES)))
    LAST_EXEC_TIME_NS = res.exec_time_ns
    out = np.concatenate([res.results[c]["out_shard"] for c in range(NCORES)],
                         axis=0)
    return out.reshape(B, N, D).astype(np.float32)



# revision 6
# speedup vs baseline: 1.0668x; 1.0668x over previous
"""Distributed Trainium2 kernel: LayerNorm -> QKV -> causal MHA -> out-proj.

Sharding (8 cores):
  - LayerNorm + final projection: token-parallel (4096 tokens -> 512/core).
  - Attention + QKV: head-parallel (16 heads -> 2/core).
  - Comms: AllGather of xn^T (bf16) after LN; AllToAll of per-head attention
    output before the final projection (re-shards the inner-dim contraction
    to token-parallel; no AllReduce needed).

Layout notes:
  - All activations are kept TRANSPOSED ([feature, token]) so every matmul
    contraction runs over the partition axis.  S is computed transposed
    (S^T[j,i] = k_j . q_i); softmax sums come from an appended ones-column
    on V (m=65 matmul); causal masking is a multiplicative 0/1 bf16 mask on
    exp(S^T) tiles; the dh^-0.5 scale rides the exp activation's free affine.
  - Matmul inputs are bf16 (4x the fp32 TensorE rate); accumulation fp32.
    Weights arrive pre-cast to bf16 (host-side layout prep).
  - Attention groups are interleaved into the QKV token-chunk loop (group
    (b, ic) right after its last prerequisite chunk) so ScalarE exp work
    overlaps TensorE QKV matmuls and the PE stream stays dense (HAM-warm).
"""

import numpy as np
import ml_dtypes

import concourse.bass as bass
import concourse.tile as tile
from concourse import bacc, mybir
from concourse.bass import ds, ts
from concourse.bass_utils import run_bass_kernel_spmd
from concourse.masks import make_identity

B, N, D = 2, 2048, 1024
HEADS, DH = 16, 64
INNER = HEADS * DH          # 1024
NCORES = 8
T = B * N                   # 4096 tokens
TS = T // NCORES            # 512 tokens per core
HPC = HEADS // NCORES       # 2 heads per core
SCALE = float(DH) ** -0.5   # 0.125
EPS = 1e-5

FP = mybir.dt.float32
BF = mybir.dt.bfloat16

KT = D // 128               # 8 contraction tiles of 128 over dim
TCH = T // 512              # 8 token chunks of 512 (== ranks)
ICB = N // 512              # 4 i-chunks of 512 per batch
JTB = N // 128              # 16 j-tiles of 128 per batch


def build():
    nc = bacc.Bacc("TRN2", target_bir_lowering=False, debug=False,
                   num_devices=NCORES)

    x_sh = nc.dram_tensor("x_shard", [TS, D], FP, kind="ExternalInput")
    gamma_t = nc.dram_tensor("gamma", [D], FP, kind="ExternalInput")
    beta_t = nc.dram_tensor("beta", [D], FP, kind="ExternalInput")
    wq_t = nc.dram_tensor("wq", [D, HPC * DH], BF, kind="ExternalInput")
    wk_t = nc.dram_tensor("wk", [D, HPC * DH], BF, kind="ExternalInput")
    wv_t = nc.dram_tensor("wv", [D, HPC * DH], BF, kind="ExternalInput")
    wo_t = nc.dram_tensor("w_out", [INNER, D], BF, kind="ExternalInput")
    out_sh = nc.dram_tensor("out_shard", [TS, D], FP, kind="ExternalOutput")

    with tile.TileContext(nc) as tc:
        _body(nc, tc, x_sh, gamma_t, beta_t, wq_t, wk_t, wv_t, wo_t, out_sh)

    nc.compile()
    return nc


def _att_thunks(nc, b, ic, kTt, qT, vhat, tri, outT,
                s_ps, av_ps, espool, smallp):
    """Attention for query chunk (b, ic) as a list of PE-ordered thunks.

    Ordering per jp step emits the two heads' S matmuls back-to-back so the
    row-disjoint (tile_position 0 / 64) K=64 matmuls co-execute on the PE
    array.  Streams are causally trimmed: for a diagonal j-tile (offset m in
    the 512-block), only queries i >= 128*m are computed/streamed; the one
    remaining partial 128x128 diagonal tile is zeroed with a multiplicative
    triangle mask after exp.  S-matmuls for step jp are emitted before the
    AV-matmuls of step jp-1 so the PE never waits on the ACT exp latency.
    """
    q_idx = b * ICB + ic
    njt = 4 * (ic + 1)
    av = [av_ps.tile([128, 512], FP, tag=f"av{h}", name=f"av{h}_{q_idx}")
          for h in range(HPC)]
    es = {}

    def i0_of(jt):
        m = jt - 4 * ic
        return 128 * m if m > 0 else 0

    def s_mm(h, jp, u):
        def run():
            if u == 0:
                sx = s_ps.tile([128, 1024], FP, tag="sx",
                               name=f"sx{h}_{q_idx}_{jp}")
                es[(h, jp)] = (sx, None)
            sx, _ = es[(h, jp)]
            jt = 2 * jp + u
            tq = b * ICB + jt // 4
            jo = 128 * (jt % 4)
            i0 = i0_of(jt)
            nc.tensor.matmul(
                sx[:, ds(512 * u + i0, 512 - i0)],
                kTt[ds(64 * h, 64), tq, ds(jo, 128)],
                qT[ds(64 * h, 64), q_idx, ds(i0, 512 - i0)],
                start=True, stop=True,
                tile_position=(64 * h, 0))
        return run

    def exp_t(h, jp):
        def run():
            sx, _ = es[(h, jp)]
            e = espool.tile([128, 1024], BF, tag="es",
                            name=f"es{h}_{q_idx}_{jp}")
            es[(h, jp)] = (sx, e)
            i0s = [i0_of(2 * jp), i0_of(2 * jp + 1)]
            if i0s[0] == 0 and i0s[1] == 0:
                nc.scalar.activation(
                    out=e, in_=sx,
                    func=mybir.ActivationFunctionType.Exp, scale=SCALE)
            else:
                for u in range(2):
                    i0 = i0s[u]
                    nc.scalar.activation(
                        out=e[:, ds(512 * u + i0, 512 - i0)],
                        in_=sx[:, ds(512 * u + i0, 512 - i0)],
                        func=mybir.ActivationFunctionType.Exp, scale=SCALE)
        return run

    def av_mm(h, jp, u):
        def run():
            _, e = es[(h, jp)]
            jt = 2 * jp + u
            m = jt - 4 * ic
            i0 = i0_of(jt)
            if 0 <= m < 4:
                # zero the strict upper triangle of the diagonal 128x128 tile
                nc.vector.tensor_tensor(
                    out=e[:, ds(512 * u + 128 * m, 128)],
                    in0=e[:, ds(512 * u + 128 * m, 128)],
                    in1=tri,
                    op=mybir.AluOpType.mult)
            nc.tensor.matmul(
                av[h][0:65, ds(i0, 512 - i0)],
                vhat[:, b * JTB + jt, ds(65 * h, 65)],
                e[:, ds(512 * u + i0, 512 - i0)],
                start=(jt == 0), stop=(jt == njt - 1))
        return run

    def norm(h):
        def run():
            rsum = smallp.tile([1, 512], FP, tag="rsum", name=f"rs{h}_{q_idx}")
            nc.vector.tensor_copy(out=rsum, in_=av[h][64:65, :])
            rec = smallp.tile([1, 512], FP, tag="rec", name=f"rc{h}_{q_idx}")
            nc.vector.reciprocal_approx_fast(out=rec, in_=rsum)
            bc = smallp.tile([64, 512], FP, tag="bc", name=f"bc{h}_{q_idx}")
            nc.gpsimd.partition_broadcast(bc, rec)
            nc.vector.tensor_tensor(
                out=outT[h][:, ds(512 * q_idx, 512)],
                in0=av[h][0:64, :], in1=bc,
                op=mybir.AluOpType.mult)
        return run

    def s_group(jp):
        return [s_mm(0, jp, 0), s_mm(1, jp, 0),
                s_mm(0, jp, 1), s_mm(1, jp, 1),
                exp_t(0, jp), exp_t(1, jp)]

    def av_group(jp):
        return [av_mm(0, jp, 0), av_mm(0, jp, 1),
                av_mm(1, jp, 0), av_mm(1, jp, 1)]

    thunks = []
    nps = njt // 2
    thunks.extend(s_group(0))
    for jp in range(1, nps):
        thunks.extend(s_group(jp))
        thunks.extend(av_group(jp - 1))
    thunks.extend(av_group(nps - 1))
    thunks.append(norm(0))
    thunks.append(norm(1))
    return thunks


def _qkv_thunks(nc, tci, xt_full, wq_sb, wk_sb, wv_sb, qT, kTt, vhat,
                identity, qkv_ps, s_ps, vst):
    """QKV projection for token chunk tci as a list of PE-ordered thunks."""
    thunks = []

    def mk_group(w_sb, nm):
        acc = qkv_ps.tile([128, 512], FP, tag="acc", name=f"acc{nm}_{tci}")

        def mm(k):
            def run():
                nc.tensor.matmul(acc, w_sb[:, k, :], xt_full[:, k, :],
                                 start=(k == 0), stop=(k == KT - 1))
            return run
        return acc, mm

    for w_sb, dst, nm in ((wq_sb, qT, "q"), (wk_sb, kTt, "k")):
        acc, mm = mk_group(w_sb, nm)
        for k in range(KT):
            thunks.append(mm(k))

        def copy(acc=acc, dst=dst, tci=tci):
            nc.vector.tensor_copy(out=dst[:, tci, :], in_=acc)
        thunks.append(copy)

    accv, mmv = mk_group(wv_sb, "v")
    for k in range(KT):
        thunks.append(mmv(k))
    vs = vst.tile([128, 512], BF, tag="vs", name=f"vs_{tci}")

    def vcopy():
        nc.vector.tensor_copy(out=vs, in_=accv)
    thunks.append(vcopy)

    def vtrans():
        def run():
            vstg = vst.tile([128, 4, 128], BF, tag="vstg", name=f"vstg_{tci}")
            nc.sync.dma_start_transpose(out=vstg, in_=vs)
            nc.vector.tensor_copy(out=vhat[:, ds(tci * 4, 4), 0:64],
                                  in_=vstg[:, :, 0:64])
            nc.vector.tensor_copy(out=vhat[:, ds(tci * 4, 4), 65:129],
                                  in_=vstg[:, :, 64:128])
        return run
    thunks.append(vtrans())
    return thunks


def _merge(primary, filler):
    """Interleave filler thunks evenly between primary thunks."""
    out = []
    np_, nf = len(primary), len(filler)
    fi = 0
    for i, p in enumerate(primary):
        out.append(p)
        want = (i + 1) * nf // np_
        while fi < want:
            out.append(filler[fi])
            fi += 1
    out.extend(filler[fi:])
    return out


def _body(nc, tc, x_sh, gamma_t, beta_t, wq_t, wk_t, wv_t, wo_t, out_sh):
    from contextlib import ExitStack
    ctx = ExitStack()
    with ctx:
        const = ctx.enter_context(tc.tile_pool(name="const", bufs=1))
        wpool = ctx.enter_context(tc.tile_pool(name="wpool", bufs=1))
        big = ctx.enter_context(tc.tile_pool(name="big", bufs=1))
        dram = ctx.enter_context(tc.tile_pool(name="dram", bufs=1, space="DRAM"))

        # ---------- constants ----------
        identity = const.tile([128, 128], BF)
        make_identity(nc, identity)

        # 0/1 lower-triangle mask for the one partial 128x128 diagonal tile
        # of S^T: keep [jj, ii] iff ii - jj >= 0.
        tri = const.tile([128, 128], BF)
        nc.gpsimd.memset(tri, 1.0)
        nc.gpsimd.affine_select(
            out=tri, in_=tri,
            compare_op=mybir.AluOpType.is_ge, fill=0.0,
            base=0, pattern=[[1, 128]], channel_multiplier=-1,
        )

        eps_t = const.tile([128, 1], FP)
        nc.vector.memset(eps_t, EPS)

        gamma_b = const.tile([128, D], FP)
        g_ap = gamma_t.ap()
        nc.sync.dma_start(out=gamma_b, in_=bass.AP(
            tensor=g_ap.tensor, offset=g_ap.offset,
            ap=[[0, 128]] + list(g_ap.ap)))
        beta_b = const.tile([128, D], BF)
        beta_f = const.tile([128, D], FP)
        b_ap = beta_t.ap()
        nc.sync.dma_start(out=beta_f, in_=bass.AP(
            tensor=b_ap.tensor, offset=b_ap.offset,
            ap=[[0, 128]] + list(b_ap.ap)))
        nc.vector.tensor_copy(out=beta_b, in_=beta_f)

        # ---------- comm bounce buffers ----------
        ag_in = dram.tile([KT, 128, TS], BF)
        ag_out = dram.tile([NCORES, KT, 128, TS], BF, addr_space="Shared")
        a2a_in = dram.tile([NCORES, 128, TS], BF)
        a2a_out = dram.tile([NCORES, 128, TS], BF)

        wq_sb = wpool.tile([128, KT, HPC * DH], BF)
        wk_sb = wpool.tile([128, KT, HPC * DH], BF)
        wv_sb = wpool.tile([128, KT, HPC * DH], BF)
        wo_sb = wpool.tile([128, KT, D], BF)

        # ---------- phase A: LayerNorm + transpose -> ag_in ----------
        with tc.tile_pool(name="lnp", bufs=2) as lnp, \
             tc.tile_pool(name="lns", bufs=2) as lns, \
             tc.tile_pool(name="tstage", bufs=4) as tstage:
            nc.sync.dma_start(
                out=wq_sb,
                in_=wq_t.ap().rearrange("(k p) c -> p k c", p=128))
            nc.sync.dma_start(
                out=wk_sb,
                in_=wk_t.ap().rearrange("(k p) c -> p k c", p=128))
            nc.sync.dma_start(
                out=wv_sb,
                in_=wv_t.ap().rearrange("(k p) c -> p k c", p=128))
            x_tiles = []
            for tt in range(TS // 128):
                x_t = lnp.tile([128, D], FP, tag=f"x{tt}", name=f"x_{tt}",
                               bufs=1)
                nc.sync.dma_start(out=x_t, in_=x_sh.ap()[ts(tt, 128), :])
                x_tiles.append(x_t)
            for tt in range(TS // 128):
                x_t = x_tiles[tt]
                stats = lns.tile([128, 2, 6], FP, tag="stats")
                xg = x_t.rearrange("p (s f) -> p s f", f=512)
                for s in range(2):
                    nc.vector.bn_stats(out=stats[:, s, :], in_=xg[:, s, :])
                mv = lns.tile([128, 2], FP, tag="mv")
                nc.vector.bn_aggr(out=mv, in_=stats)
                rstd = lns.tile([128, 1], FP, tag="rstd")
                nc.scalar.activation(out=rstd, in_=mv[:, 1:2],
                                     func=mybir.ActivationFunctionType.Sqrt,
                                     bias=eps_t, scale=1.0)
                nc.vector.reciprocal(out=rstd, in_=rstd)
                xn_f = lnp.tile([128, D], FP, tag="xnf")
                nc.vector.tensor_scalar(
                    out=xn_f, in0=x_t, scalar1=mv[:, 0:1], scalar2=rstd,
                    op0=mybir.AluOpType.subtract, op1=mybir.AluOpType.mult)
                xn_g = lnp.tile([128, D], BF, tag="xng")
                nc.vector.tensor_tensor(out=xn_g, in0=xn_f, in1=gamma_b,
                                        op=mybir.AluOpType.mult)
                xn_bf = lnp.tile([128, D], BF, tag="xnbf")
                nc.vector.tensor_tensor(out=xn_bf, in0=xn_g, in1=beta_b,
                                        op=mybir.AluOpType.add)
                xnT_s = tstage.tile([128, KT, 128], BF, tag="xnT")
                nc.scalar.dma_start_transpose(out=xnT_s, in_=xn_bf)
                nc.sync.dma_start(
                    out=ag_in[:, :, ts(tt, 128)].rearrange("k p t -> p k t"),
                    in_=xnT_s)

        nc.gpsimd.collective_compute(
            "AllGather", mybir.AluOpType.bypass,
            replica_groups=[list(range(NCORES))],
            ins=[ag_in.opt()], outs=[ag_out.opt()])

        # ---------- phase B+C: QKV projection + interleaved attention ----
        qT = big.tile([128, TCH, 512], BF)   # rows: [h0 64 | h1 64]
        kTt = big.tile([128, TCH, 512], BF)
        vhat = big.tile([128, JTB * B, 130], BF)  # [j, jt, 65*h+c], col 64/129 = 1
        nc.gpsimd.memset(vhat[:, :, 64:65], 1.0)
        nc.gpsimd.memset(vhat[:, :, 129:130], 1.0)
        outT = [big.tile([64, T], BF, name=f"outT{h}") for h in range(HPC)]

        with tc.tile_pool(name="xstream", bufs=3) as xstream, \
             tc.tile_pool(name="qkv_ps", bufs=2, space="PSUM") as qkv_ps, \
             tc.tile_pool(name="s_ps", bufs=2, space="PSUM") as s_ps, \
             tc.tile_pool(name="av_ps", bufs=1, space="PSUM") as av_ps, \
             tc.tile_pool(name="espool", bufs=6) as espool, \
             tc.tile_pool(name="smallp", bufs=4) as smallp, \
             tc.tile_pool(name="vstage", bufs=2) as vst:
            pending_att = None
            for tci in range(TCH):
                xt_full = xstream.tile([128, KT, 512], BF, tag="xt",
                                       name=f"xt_{tci}")
                nc.sync.dma_start(
                    out=xt_full,
                    in_=ag_out[tci].rearrange("k p t -> p k t"))
                if tci == 1:
                    nc.sync.dma_start(
                        out=wo_sb,
                        in_=wo_t.ap().rearrange("(k p) e -> p k e", p=128))
                qkv = _qkv_thunks(nc, tci, xt_full, wq_sb, wk_sb, wv_sb,
                                  qT, kTt, vhat, identity, qkv_ps, s_ps, vst)
                if pending_att is None:
                    seq = qkv
                else:
                    seq = _merge(pending_att, qkv)
                for thunk in seq:
                    thunk()
                b, ic = tci // ICB, tci % ICB
                pending_att = _att_thunks(nc, b, ic, kTt, qT, vhat, tri,
                                          outT, s_ps, av_ps, espool, smallp)
            for thunk in pending_att:
                thunk()

        # ---------- phase D: AllToAll + output projection ----------
        for r in range(NCORES):
            nc.sync.dma_start(out=a2a_in[r, 0:64, :],
                              in_=outT[0][:, ds(512 * r, 512)])
            nc.sync.dma_start(out=a2a_in[r, 64:128, :],
                              in_=outT[1][:, ds(512 * r, 512)])
        nc.gpsimd.collective_compute(
            "AllToAll", mybir.AluOpType.bypass,
            replica_groups=[list(range(NCORES))],
            ins=[a2a_in.opt()], outs=[a2a_out.opt()])

        a2a_sb = big.tile([128, NCORES, 512], BF)
        for r in range(NCORES):
            nc.sync.dma_start(out=a2a_sb[:, r, :], in_=a2a_out[r])

        out_view = out_sh.ap().rearrange("(t p) e -> p t e", p=128)
        with tc.tile_pool(name="op_ps", bufs=4, space="PSUM") as op_ps, \
             tc.tile_pool(name="ost", bufs=3) as ostp:
            for tt in range(TS // 128):
                for ec in range(D // 512):
                    po = op_ps.tile([128, 512], FP, tag="po")
                    for ct in range(NCORES):
                        nc.tensor.matmul(
                            po, a2a_sb[:, ct, ds(128 * tt, 128)],
                            wo_sb[:, ct, ds(512 * ec, 512)],
                            start=(ct == 0), stop=(ct == NCORES - 1))
                    ost = ostp.tile([128, 512], FP, tag="ost")
                    nc.vector.tensor_copy(out=ost, in_=po)
                    nc.sync.dma_start(out=out_view[:, tt, ds(512 * ec, 512)],
                                      in_=ost)


_NC = None
LAST_EXEC_TIME_NS = None


def _get_nc():
    global _NC
    if _NC is None:
        _NC = build()
    return _NC


def make_in_maps(x, gamma, beta, w_qkv, w_out):
    bf = ml_dtypes.bfloat16
    x = np.ascontiguousarray(np.asarray(x, dtype=np.float32)).reshape(T, D)
    gamma = np.ascontiguousarray(np.asarray(gamma, dtype=np.float32))
    beta = np.ascontiguousarray(np.asarray(beta, dtype=np.float32))
    w_qkv = np.asarray(w_qkv, dtype=np.float32).astype(bf)
    w_out = np.ascontiguousarray(np.asarray(w_out, dtype=np.float32).astype(bf))
    in_maps = []
    for c in range(NCORES):
        cols = slice(128 * c, 128 * c + 128)
        in_maps.append({
            "x_shard": np.ascontiguousarray(x[TS * c: TS * (c + 1)]),
            "gamma": gamma,
            "beta": beta,
            "wq": np.ascontiguousarray(w_qkv[:, cols]),
            "wk": np.ascontiguousarray(w_qkv[:, INNER:][:, cols]),
            "wv": np.ascontiguousarray(w_qkv[:, 2 * INNER:][:, cols]),
            "w_out": w_out,
        })
    return in_maps


def kernel(x, mask, gamma, beta, w_qkv, w_out):
    global LAST_EXEC_TIME_NS
    nc = _get_nc()
    in_maps = make_in_maps(x, gamma, beta, w_qkv, w_out)
    res = run_bass_kernel_spmd(nc, in_maps, core_ids=list(range(NCORES)))
    LAST_EXEC_TIME_NS = res.exec_time_ns
    out = np.concatenate([res.results[c]["out_shard"] for c in range(NCORES)],
                         axis=0)
    return out.reshape(B, N, D).astype(np.float32)

